# revision 20
# baseline (speedup 1.0000x reference)
import numpy as np
import ml_dtypes

B, S, I, H, O = 64, 512, 256, 512, 256
NCORES = 8
BL = B // NCORES
P = 128
KI, KH, KO = I // P, H // P, O // P
CH = 512
_builder_cache = {}

REC_ORDER = [(2, 2), (0, 3), (3, 3), (0, 2), (2, 3), (3, 2), (2, 0), (2, 1),
             (3, 0), (3, 1), (1, 1), (1, 0), (0, 1), (1, 2), (0, 0), (1, 3)]
GTANH_POS = {"Q": 9, "P": 15}
GROUP_OF = {0: "P", 1: "P", 2: "Q", 3: "Q"}


def build_nc(seq_len=S):
    import concourse.bass as bass
    import concourse.mybir as mybir
    import concourse.tile as tile
    from concourse import bacc
    from concourse.tile import add_dep_helper

    f32 = mybir.dt.float32
    bf16 = mybir.dt.bfloat16
    AF = mybir.ActivationFunctionType

    s = seq_len
    assert s % 2 == 0
    F = s * BL
    nch = max(1, F // CH)
    ch = F // nch
    F2 = F // 2
    CH2 = 256
    nch2 = max(1, F2 // CH2)
    ch2 = F2 // nch2

    nc = bacc.Bacc("TRN2")

    xt = nc.dram_tensor("xt", [I, F], bf16, kind="ExternalInput")
    h0t = nc.dram_tensor("h0t", [H, BL], bf16, kind="ExternalInput")
    wih = nc.dram_tensor("wih", [I, H], bf16, kind="ExternalInput")
    whh = nc.dram_tensor("whh", [H, H], bf16, kind="ExternalInput")
    wff = nc.dram_tensor("wff", [H, O], bf16, kind="ExternalInput")
    bcb = nc.dram_tensor("bcb", [P, KH], f32, kind="ExternalInput")
    bfb = nc.dram_tensor("bfb", [P, KO], f32, kind="ExternalInput")
    y = nc.dram_tensor("y", [KO, P, 2, F2], f32, kind="ExternalOutput")

    with tile.TileContext(nc) as tc:
        with (
            tc.tile_pool(name="const", bufs=1) as cp,
            tc.tile_pool(name="big", bufs=1) as bp,
        ):
            wih_sb = cp.tile([P, KI, H], bf16)
            whh_sb = cp.tile([P, KH, H], bf16)
            wff_sb = cp.tile([P, KH, O], bf16)
            bcb_sb = cp.tile([P, KH], f32)
            bfb_sb = cp.tile([P, KO], f32)
            scr_sb = cp.tile([P, P], bf16)

            xt_sb = bp.tile([P, KI, F], bf16)
            xp_sb = bp.tile([P, KH, F], bf16)
            n0 = (s // 2 + 1) * BL
            n1 = (s // 2) * BL
            hsP0 = bp.tile([P, 2, n0], bf16)
            hsQ0 = bp.tile([P, 2, n0], bf16)
            hsP1 = bp.tile([P, 2, n1], bf16)
            hsQ1 = bp.tile([P, 2, n1], bf16)
            hP = [hsP0, hsP1]
            hQ = [hsQ0, hsQ1]
            out_sb = bp.tile([P, KO, 2, F2], f32)

            xt_r = xt[:].rearrange("(k p) f -> p k f", p=P)
            nc.vector.memset(scr_sb[:], 1.0)
            nc.sync.dma_start(xt_sb[:, :, 0:ch], xt_r[:, :, 0:ch])
            nc.sync.dma_start(wih_sb[:], wih[:].rearrange("(k p) h -> p k h", p=P))
            nc.sync.dma_start(bcb_sb[:], bcb[:])
            h0r = h0t[:].rearrange("(k p) b -> p k b", p=P)
            nc.sync.dma_start(hsP0[:, :, 0:BL], h0r[:, 0:2, :])
            nc.sync.dma_start(hsQ0[:, :, 0:BL], h0r[:, 2:KH, :])
            nc.sync.dma_start(whh_sb[:], whh[:].rearrange("(k p) h -> p k h", p=P))
            nc.sync.dma_start(wff_sb[:], wff[:].rearrange("(k p) o -> p k o", p=P))
            nc.sync.dma_start(bfb_sb[:], bfb[:])

            for j in range(1, nch):
                slj = slice(j * ch, (j + 1) * ch)
                nc.sync.dma_start(xt_sb[:, :, slj], xt_r[:, :, slj])
            with tc.tile_pool(name="g1ps", bufs=4, space=bass.MemorySpace.PSUM) as g1p:
                wm = g1p.tile([P, 512], f32, tag="ps")
                for _ in range(32):
                    nc.tensor.matmul(
                        wm[:, 0:P], scr_sb[:], scr_sb[:], start=True, stop=True
                    )
                sl = slice(0, ch)
                for m in range(KH):
                    ps = g1p.tile([P, ch], f32)
                    for k in range(KI):
                        nc.tensor.matmul(
                            ps[:],
                            wih_sb[:, k, m * P : (m + 1) * P],
                            xt_sb[:, k, sl],
                            start=(k == 0),
                            stop=(k == KI - 1),
                        )
                    if m % 2 == 0:
                        nc.vector.tensor_scalar_add(
                            xp_sb[:, m, sl], ps[:], bcb_sb[:, m : m + 1]
                        )
                    else:
                        nc.scalar.activation(
                            xp_sb[:, m, sl], ps[:], AF.Identity,
                            bias=bcb_sb[:, m : m + 1],
                        )

            y_r = y[:].rearrange("o p q f -> p o q f")
            with (
                tc.tile_pool(name="zPp", bufs=2, space=bass.MemorySpace.PSUM) as zPp,
                tc.tile_pool(name="zQp", bufs=2, space=bass.MemorySpace.PSUM) as zQp,
                tc.tile_pool(name="g1lp", bufs=2, space=bass.MemorySpace.PSUM) as g1lp,
                tc.tile_pool(name="g2lp", bufs=2, space=bass.MemorySpace.PSUM) as g2lp,
            ):
                gpools = {"P": zPp, "Q": zQp}
                for gname, gp in gpools.items():
                    for _ in range(2):
                        zpr = gp.tile([P, 2, 256], f32, name=f"zpr{gname}",
                                      tag=f"z{gname}")
                        for half in range(2):
                            nc.tensor.matmul(
                                zpr[:, half, 0:BL],
                                whh_sb[:, 0, 0:P],
                                whh_sb[:, 0, 0:BL],
                                start=(half == 0), stop=(half == 1),
                                skip_group_check=True,
                            )

                prev_mm = None
                prev_act = None
                last_pos = {m: max(i for i, (mm_, _) in enumerate(REC_ORDER)
                                   if mm_ == m) for m in range(4)}

                pe_extras = {}
                dve_extras = {}
                dma_extras = {}

                g1_state = {}

                def g1_mm(j, m, k, half):
                    def f():
                        if (j, m) not in g1_state:
                            g1_state[(j, m)] = g1lp.tile(
                                [P, ch], f32, name="g1t", tag="g1t")
                        ps = g1_state[(j, m)]
                        h0_ = half * (ch // 2)
                        h1_ = (half + 1) * (ch // 2)
                        return nc.tensor.matmul(
                            ps[:, h0_:h1_],
                            wih_sb[:, k, m * P : (m + 1) * P],
                            xt_sb[:, k, j * ch + h0_ : j * ch + h1_],
                            start=(k == 0),
                            stop=(k == KI - 1),
                            skip_group_check=True,
                        )
                    return f

                def g1_drain(j, m, half):
                    def f():
                        ps = g1_state[(j, m)]
                        hs = slice(half * (ch // 2), (half + 1) * (ch // 2))
                        slj = slice(j * ch + half * (ch // 2),
                                    j * ch + (half + 1) * (ch // 2))
                        nc.vector.tensor_scalar_add(
                            xp_sb[:, m, slj], ps[:, hs], bcb_sb[:, m : m + 1]
                        )
                    return f

                for u in range(4 * (nch - 1)):
                    j, m = 1 + u // 4, u % 4
                    base_t = 2 + 6 * u
                    for q_ in range(4):
                        pe_extras.setdefault(base_t + q_, []).append(
                            g1_mm(j, m, q_ % 2, q_ // 2))
                    dve_extras.setdefault(base_t + 4, []).append(g1_drain(j, m, 0))
                    dve_extras.setdefault(base_t + 5, []).append(g1_drain(j, m, 1))

                g2_state = {}

                def g2_mm(c0, c1, par, ot, k, half, nhalf):
                    def f():
                        key = (c0, par, ot)
                        if key not in g2_state:
                            g2_state[key] = g2lp.tile(
                                [P, ch2], f32, name="g2t", tag="g2t")
                        ps = g2_state[key]
                        base = BL if par == 0 else 0
                        w_ = (c1 - c0) // nhalf
                        rsrc = hP[par] if k < 2 else hQ[par]
                        return nc.tensor.matmul(
                            ps[:, half * w_ : (half + 1) * w_],
                            wff_sb[:, k, ot * P : (ot + 1) * P],
                            rsrc[:, k % 2,
                                 base + c0 + half * w_ : base + c0 + (half + 1) * w_],
                            start=(k == 0),
                            stop=(k == KH - 1),
                            skip_group_check=True,
                        )
                    return f

                def g2_drain(c0, c1, par, ot):
                    def f():
                        ps = g2_state[(c0, par, ot)]
                        nc.vector.tensor_scalar_add(
                            out_sb[:, ot, par, c0:c1], ps[:, 0 : c1 - c0],
                            bfb_sb[:, ot : ot + 1]
                        )
                    return f

                def g2_dma(c0, c1, par):
                    def f():
                        nc.sync.dma_start(
                            y_r[:, :, par, c0:c1], out_sb[:, :, par, c0:c1]
                        )
                    return f

                g2_units = [(j2 * ch2, (j2 + 1) * ch2) for j2 in range(nch2 - 1)]
                g2_units.append(((nch2 - 1) * ch2, (nch2 - 1) * ch2 + ch2 // 2))
                g2_done = set()
                start_t = 0
                for (c0, c1) in g2_units:
                    ready = 2 * (c1 // BL) + 2
                    for par in range(2):
                        for ot in range(KO):
                            start_t = max(ready, start_t + 10)
                            if start_t + 9 >= s:
                                continue
                            for half in range(2):
                                for k in range(KH):
                                    pe_extras.setdefault(
                                        start_t + 4 * half + k, []).append(
                                        g2_mm(c0, c1, par, ot, k, half, 2))
                            dve_extras.setdefault(start_t + 8, []).append(
                                g2_drain(c0, c1, par, ot))
                            if ot == KO - 1:
                                dma_extras.setdefault(start_t + 9, []).append(
                                    g2_dma(c0, c1, par))
                            g2_done.add((c0, c1, par, ot))

                for t in range(s):
                    rP, rQ = hP[t % 2], hQ[t % 2]
                    wP, wQ = hP[(t + 1) % 2], hQ[(t + 1) % 2]
                    rof = (t // 2) * BL
                    wof = ((t + 1) // 2) * BL

                    zg = {g: gpools[g].tile([P, 2, 256], f32, name=f"z{g}_t",
                                            tag=f"z{g}") for g in ("P", "Q")}
                    for f in pe_extras.get(t, []):
                        mm = f()
                        if prev_mm is not None:
                            add_dep_helper(mm.ins, prev_mm.ins, sync=False)
                        prev_mm = mm
                    nc.vector.tensor_scalar_add(
                        zg["Q"][:, :, 0:BL],
                        xp_sb[:, 2:KH, t * BL : (t + 1) * BL],
                        0.0,
                    )
                    nc.vector.tensor_scalar_add(
                        zg["P"][:, :, 0:BL],
                        xp_sb[:, 0:2, t * BL : (t + 1) * BL],
                        0.0,
                    )
                    for f in dve_extras.get(t, []):
                        f()
                    for f in dma_extras.get(t, []):
                        f()

                    for i, (m, k) in enumerate(REC_ORDER):
                        rsrc = rP if k < 2 else rQ
                        rhs = rsrc[:, k % 2, rof : rof + BL]
                        zt = zg[GROUP_OF[m]]
                        mm = nc.tensor.matmul(
                            zt[:, m % 2, 0:BL],
                            whh_sb[:, k, m * P : (m + 1) * P],
                            rhs,
                            start=False,
                            stop=(i == last_pos[m]),
                            skip_group_check=True,
                        )
                        if prev_mm is not None:
                            add_dep_helper(mm.ins, prev_mm.ins, sync=False)
                        prev_mm = mm
                        if GTANH_POS["Q"] == i:
                            act = nc.scalar.activation(
                                wQ[:, :, wof : wof + BL], zg["Q"][:, :, 0:BL],
                                AF.Tanh,
                            )
                        elif GTANH_POS["P"] == i:
                            act = nc.scalar.activation(
                                wP[:, :, wof : wof + BL], zg["P"][:, :, 0:BL],
                                AF.Tanh,
                            )
                        else:
                            continue
                        if prev_act is not None:
                            add_dep_helper(act.ins, prev_act.ins, sync=False)
                        prev_act = act

            tail_ranges = list(g2_units)
            pos = max(c1 for (_, c1) in g2_units)
            if pos < F2:
                tail_ranges.append((pos, F2))
            with tc.tile_pool(name="g2ps", bufs=4,
                              space=bass.MemorySpace.PSUM) as g2p:
                nd = 0
                for (c0, c1) in tail_ranges:
                    for par in range(2):
                        for ot in range(KO):
                            if (c0, c1, par, ot) in g2_done:
                                continue
                            pP, pQ = hP[par], hQ[par]
                            base = BL if par == 0 else 0
                            ps = g2p.tile([P, ch2], f32, tag="g2ps")
                            for k in range(KH):
                                rsrc = pP if k < 2 else pQ
                                nc.tensor.matmul(
                                    ps[:, 0 : c1 - c0],
                                    wff_sb[:, k, ot * P : (ot + 1) * P],
                                    rsrc[:, k % 2, base + c0 : base + c1],
                                    start=(k == 0),
                                    stop=(k == KH - 1),
                                )
                            if nd % 2 == 0:
                                nc.vector.tensor_scalar_add(
                                    out_sb[:, ot, par, c0:c1],
                                    ps[:, 0 : c1 - c0],
                                    bfb_sb[:, ot : ot + 1]
                                )
                            else:
                                nc.scalar.activation(
                                    out_sb[:, ot, par, c0:c1],
                                    ps[:, 0 : c1 - c0], AF.Identity,
                                    bias=bfb_sb[:, ot : ot + 1],
                                )
                            nd += 1
                            if ot == KO - 1:
                                nc.sync.dma_start(
                                    y_r[:, :, par, c0:c1],
                                    out_sb[:, :, par, c0:c1]
                                )

    return nc


def make_in_maps(x, h0, W_ih, W_hh, b_ih, b_hh, W_ff, b_ff, seq_len=S):
    bf = ml_dtypes.bfloat16
    x = np.asarray(x, np.float32)
    h0 = np.asarray(h0, np.float32)
    wih = np.ascontiguousarray(np.asarray(W_ih, np.float32).T).astype(bf)
    whh = np.ascontiguousarray(np.asarray(W_hh, np.float32).T).astype(bf)
    wff = np.ascontiguousarray(np.asarray(W_ff, np.float32).T).astype(bf)
    bc = np.asarray(b_ih, np.float32) + np.asarray(b_hh, np.float32)
    bcb = np.ascontiguousarray(bc.reshape(KH, P).T)
    bfb = np.ascontiguousarray(np.asarray(b_ff, np.float32).reshape(KO, P).T)

    in_maps = []
    for c in range(NCORES):
        xs = x[c * BL : (c + 1) * BL, :seq_len]
        xt = np.ascontiguousarray(xs.transpose(2, 1, 0)).reshape(I, seq_len * BL)
        h0t = np.ascontiguousarray(h0[c * BL : (c + 1) * BL].T)
        in_maps.append(
            {
                "xt": xt.astype(bf),
                "h0t": h0t.astype(bf),
                "wih": wih,
                "whh": whh,
                "wff": wff,
                "bcb": bcb,
                "bfb": bfb,
            }
        )
    return in_maps


def assemble_output(results, seq_len=S):
    s = seq_len
    outs = []
    for r in results:
        yc = np.asarray(r["y"]).reshape(O, 2, s // 2, BL)
        full = np.empty((O, s, BL), np.float32)
        full[:, 1::2, :] = yc[:, 0]
        full[:, 0::2, :] = yc[:, 1]
        outs.append(full.transpose(2, 1, 0))
    return np.ascontiguousarray(np.concatenate(outs, axis=0))


def _get_finalized_nc(seq_len=S):
    key = ("nc", seq_len)
    if key not in _builder_cache:
        nc = build_nc(seq_len)
        nc.finalize()
        _builder_cache[key] = nc
    return _builder_cache[key]


def run_on_cores(inputs, seq_len=S, **kwargs):
    from concourse.bass_utils import run_bass_kernel_spmd

    nc = _get_finalized_nc(seq_len)
    in_maps = make_in_maps(**inputs, seq_len=seq_len)
    res = run_bass_kernel_spmd(nc, in_maps, core_ids=list(range(NCORES)), **kwargs)
    return res


def kernel(**inputs) -> np.ndarray:
    res = run_on_cores(inputs)
    return assemble_output(res.results)
